# revision 9
# baseline (speedup 1.0000x reference)
"""Trainium2 Bass kernel for nn_Detector (retrieval_knn drift detector).

Pipeline (per token):
    z1 = relu(x @ W1 + b1) @ W2 + b2
    cls = argmin_j ||z1 - centroid_j||
    z2 = relu((x+noise) @ W1 + b1) @ W2 + b2
    dis = ||z2 - centroid_cls||
    drift = |dis - med_cls| / mad_cls > 3.5

Strategy: pure data-parallel over 8 NeuronCores (8192 tokens each).
On-chip activations are feature-major ([feat, tok]) so the contraction dim
sits on partitions.  All matmuls in bf16 (output drift bits have >4.5 sigma
margin vs the 3.5 threshold, verified against the fp32 reference).

Algebraic rewrites baked in on the host:
  - argmin_j ||z1-c_j||^2 == argmax_j (z1.c_j - 0.5||c_j||^2).  The per-j
    constant (300 - 0.5||c_j||^2 + b2.c_j) is added via a rank-1 matmul
    preload into PSUM (ones (x) pre_j), split hi/lo bf16 for precision,
    which also folds away the b2 bias of the first encoder pass.
  - drift = (d2 > A_cls) | (d2 < B_cls) with d2 = ||z2' - (c_cls - b2)||^2,
    z2' the bias-free second encoding, A = (med+3.5*mad)^2 and
    B = (med-3.5*mad)^2 if med > 3.5*mad else -1.  No sqrt, no division,
    no med/mad gathers.
  - gather table rows [c_j - b2 (128 f32), A_j, B_j, pad, pad] fetched by
    one indirect DMA per 512-token tile.
"""

import numpy as np
import ml_dtypes

import concourse.bass as bass
import concourse.bacc as bacc
import concourse.mybir as mybir
import concourse.tile as tile
from concourse.masks import make_identity

BF16 = ml_dtypes.bfloat16

B, D_IN, H, D_LAT, K = 65536, 512, 256, 128, 1000
MAD_THRESHOLD = 3.5
N_CORES = 8
BS = B // N_CORES            # tokens per core
TOK_TILE = 512               # tokens per pipeline tile
KC1 = D_IN // 128            # 4  K-chunks for layer 1
FC1 = H // 128               # 2  feature chunks of the hidden layer
TAB_W = 132                  # gather-table row width (128 + A + B + 2 pad)
PRE_SHIFT = 300.0            # keeps the argmax scores positive (not required,
                             # but keeps |score| small for bf16 headroom)


def build_program(n_tiles=BS // TOK_TILE, enable_asserts=False,
                  debug_taps=False):
    """Build the per-core Bass program.  Returns (nc, names) where names maps
    logical tensors to dram tensor names."""
    bs = n_tiles * TOK_TILE
    nc = bacc.Bacc(
        "TRN2",
        target_bir_lowering=False,
        debug=False,
        enable_asserts=enable_asserts,
        num_devices=N_CORES,
    )
    f32, bf16, i32, u32 = (
        mybir.dt.float32, mybir.dt.bfloat16, mybir.dt.int32, mybir.dt.uint32,
    )

    xT = nc.dram_tensor("xT", [n_tiles, KC1, 128, TOK_TILE], bf16,
                        kind="ExternalInput").ap()
    xnT = nc.dram_tensor("xnT", [n_tiles, KC1, 128, TOK_TILE], bf16,
                         kind="ExternalInput").ap()
    W1s_d = nc.dram_tensor("W1s", [128, KC1, H], bf16, kind="ExternalInput").ap()
    W2s_d = nc.dram_tensor("W2s", [128, FC1, D_LAT], bf16,
                           kind="ExternalInput").ap()
    b1s_d = nc.dram_tensor("b1s", [128, FC1], f32, kind="ExternalInput").ap()
    cTs_d = nc.dram_tensor("cTs", [128, K], bf16, kind="ExternalInput").ap()
    pre_d = nc.dram_tensor("pre", [1, 2, K], bf16, kind="ExternalInput").ap()
    ctab = nc.dram_tensor("ctab", [K, TAB_W], f32, kind="ExternalInput").ap()
    drift_d = nc.dram_tensor("drift", [bs], i32, kind="ExternalOutput").ap()

    CH = TOK_TILE // 128     # 4 token chunks per tile
    if debug_taps:
        cls_dbg = nc.dram_tensor("cls_dbg", [n_tiles, 128, CH], u32,
                                 kind="ExternalOutput").ap()
        d2_dbg = nc.dram_tensor("d2_dbg", [n_tiles, 128, CH], f32,
                                kind="ExternalOutput").ap()
        m8_dbg = nc.dram_tensor("m8_dbg", [n_tiles, 128, CH, 8], f32,
                                kind="ExternalOutput").ap()
        tab_dbg = nc.dram_tensor("tab_dbg", [n_tiles, 128, CH, TAB_W], f32,
                                 kind="ExternalOutput").ap()

    with tile.TileContext(nc) as tc:
        with (
            tc.tile_pool(name="const", bufs=1) as const,
            tc.tile_pool(name="xin", bufs=16) as xin,
            tc.tile_pool(name="hsb", bufs=8) as hsb,
            tc.tile_pool(name="zsb", bufs=2) as zsb,
            tc.tile_pool(name="small", bufs=4) as small,
            tc.tile_pool(name="tab", bufs=2) as tabp,
            tc.tile_pool(name="acc", bufs=1) as accp,
            tc.tile_pool(name="mm", bufs=3, space="PSUM") as mmp,
            tc.tile_pool(name="gp", bufs=2, space="PSUM") as gpp,
            tc.tile_pool(name="z2r", bufs=1, space="PSUM") as z2rp,
        ):
            # ---- constants -------------------------------------------------
            W1s = const.tile([128, KC1, H], bf16)
            nc.sync.dma_start(W1s[:], W1s_d[:])
            W2s = const.tile([128, FC1, D_LAT], bf16)
            nc.sync.dma_start(W2s[:], W2s_d[:])
            b1s = const.tile([128, FC1], f32)
            nc.sync.dma_start(b1s[:], b1s_d[:])
            cTs = const.tile([128, K], bf16)
            nc.sync.dma_start(cTs[:], cTs_d[:])
            pre = const.tile([1, 2, K], bf16)
            nc.sync.dma_start(pre[:], pre_d[:])
            ones1 = const.tile([1, 128], bf16)
            nc.gpsimd.memset(ones1[:], 1.0)
            ident = const.tile([128, 128], f32)
            make_identity(nc, ident[:])

            driftacc = accp.tile([128, n_tiles * CH], f32)

            # G matmul N-halves (<=512 free dim per PSUM bank)
            halves = [(0, 512), (512, K)]

            for i in range(n_tiles):
                # ---- load inputs (feature-major bf16) ----------------------
                xts = []
                xnts = []
                for kc in range(KC1):
                    t = xin.tile([128, TOK_TILE], bf16, tag="xin")
                    nc.sync.dma_start(t[:], xT[i, kc])
                    xts.append(t)
                for kc in range(KC1):
                    t = xin.tile([128, TOK_TILE], bf16, tag="xin")
                    nc.sync.dma_start(t[:], xnT[i, kc])
                    xnts.append(t)

                # ---- layer 1 (both passes) + relu --------------------------
                h1b, h2b = [], []
                for src, dst in ((xts, h1b), (xnts, h2b)):
                    for fc in range(FC1):
                        hT = mmp.tile([128, TOK_TILE], mybir.dt.float32,
                                      tag="mm")
                        for kc in range(KC1):
                            nc.tensor.matmul(
                                hT[:],
                                lhsT=W1s[:, kc, fc * 128:(fc + 1) * 128],
                                rhs=src[kc][:],
                                start=(kc == 0),
                                stop=(kc == KC1 - 1),
                            )
                        hb = hsb.tile([128, TOK_TILE], bf16, tag="h")
                        nc.scalar.activation(
                            hb[:], hT[:], mybir.ActivationFunctionType.Relu,
                            bias=b1s[:, fc:fc + 1],
                        )
                        dst.append(hb)

                # ---- layer 2, clean pass (feature-major, bias folded) ------
                zT = mmp.tile([128, TOK_TILE], mybir.dt.float32, tag="mm")
                for kc in range(FC1):
                    nc.tensor.matmul(
                        zT[:], lhsT=W2s[:, kc, :], rhs=h1b[kc][:],
                        start=(kc == 0), stop=(kc == FC1 - 1),
                    )
                z1b = zsb.tile([128, TOK_TILE], bf16, tag="z1")
                nc.scalar.activation(z1b[:], zT[:],
                                     mybir.ActivationFunctionType.Copy)

                cls4 = small.tile([128, CH, 8], u32, tag="cls")
                d2c = small.tile([128, CH], mybir.dt.float32, tag="d2")

                for c in range(CH):
                    csl = slice(c * 128, (c + 1) * 128)

                    # ---- scores G = z1.c_j + pre_j  (PSUM, fp32) -----------
                    G = gpp.tile([128, 1024], mybir.dt.float32, tag="G")
                    for lo, hi in halves:
                        nc.tensor.matmul(
                            G[:, lo:lo + (hi - lo)],
                            lhsT=ones1[:], rhs=pre[:, 0, lo:hi],
                            start=True, stop=False,
                        )
                        nc.tensor.matmul(
                            G[:, lo:lo + (hi - lo)],
                            lhsT=ones1[:], rhs=pre[:, 1, lo:hi],
                            start=False, stop=False,
                        )
                        nc.tensor.matmul(
                            G[:, lo:lo + (hi - lo)],
                            lhsT=z1b[:, csl], rhs=cTs[:, lo:hi],
                            start=False, stop=True,
                        )

                    # ---- argmax over centroids -----------------------------
                    m8 = small.tile([128, 8], mybir.dt.float32, tag="m8")
                    nc.vector.max(out=m8[:], in_=G[:, :K])
                    nc.vector.max_index(
                        out=cls4[:, c, :], in_max=m8[:], in_values=G[:, :K],
                    )
                    if debug_taps:
                        nc.sync.dma_start(m8_dbg[i, :, c, :], m8[:])

                # ---- gather [c_j - b2, A, B, pad] rows by cls --------------
                # One indirect DMA per 128-token chunk with [128, 1] offsets:
                # multi-offset-per-partition gathers pair offsets to output
                # rows in a different order on HW than in the simulator.
                clsc = small.tile([128, CH], u32, tag="clsc")
                tabsel = tabp.tile([128, CH, TAB_W], mybir.dt.float32,
                                   tag="tab")
                for c in range(CH):
                    nc.vector.tensor_scalar(
                        out=clsc[:, c:c + 1], in0=cls4[:, c, 0:1],
                        scalar1=K - 1, scalar2=None, op0=mybir.AluOpType.min,
                    )
                    nc.gpsimd.indirect_dma_start(
                        out=tabsel[:, c, :],
                        out_offset=None,
                        in_=ctab[:],
                        in_offset=bass.IndirectOffsetOnAxis(
                            ap=clsc[:, c:c + 1], axis=0),
                    )

                for c in range(CH):
                    csl = slice(c * 128, (c + 1) * 128)
                    z2r = z2rp.tile([128, D_LAT], mybir.dt.float32, tag="z2r")
                    for kc in range(FC1):
                        nc.tensor.matmul(
                            z2r[:], lhsT=h2b[kc][:, csl], rhs=W2s[:, kc, :],
                            start=(kc == 0), stop=(kc == FC1 - 1),
                        )
                    # diff = z2r - csel   (bf16)
                    diff = small.tile([128, D_LAT], bf16, tag="diff")
                    nc.vector.scalar_tensor_tensor(
                        out=diff[:],
                        in0=tabsel[:, c, 0:D_LAT],
                        scalar=-1.0,
                        in1=z2r[:],
                        op0=mybir.AluOpType.mult,
                        op1=mybir.AluOpType.add,
                    )
                    # d2 = sum(diff*diff)
                    junk = small.tile([128, D_LAT], bf16, tag="junk")
                    nc.vector.scalar_tensor_tensor(
                        out=junk[:],
                        in0=diff[:],
                        scalar=0.0,
                        in1=diff[:],
                        op0=mybir.AluOpType.add,
                        op1=mybir.AluOpType.mult,
                        accum_out=d2c[:, c:c + 1],
                    )

                # ---- drift = (d2 > A) | (d2 < B) ---------------------------
                ga = small.tile([128, CH], mybir.dt.float32, tag="ga")
                gb = small.tile([128, CH], mybir.dt.float32, tag="gb")
                nc.vector.tensor_tensor(
                    out=ga[:], in0=d2c[:], in1=tabsel[:, :, 128],
                    op=mybir.AluOpType.is_gt,
                )
                nc.vector.tensor_tensor(
                    out=gb[:], in0=d2c[:], in1=tabsel[:, :, 129],
                    op=mybir.AluOpType.is_lt,
                )
                nc.vector.tensor_tensor(
                    out=driftacc[:, i * CH:(i + 1) * CH],
                    in0=ga[:], in1=gb[:], op=mybir.AluOpType.max,
                )
                if debug_taps:
                    nc.sync.dma_start(cls_dbg[i], clsc[:])
                    nc.sync.dma_start(d2_dbg[i], d2c[:])
                    nc.sync.dma_start(tab_dbg[i], tabsel[:])

            # ---- transpose [128, n_tiles*CH] -> token order and store ------
            ncols = n_tiles * CH
            tpsum = z2rp.tile([128, 128], mybir.dt.float32, tag="z2r")
            nc.tensor.transpose(tpsum[:ncols, :], driftacc[:, :ncols],
                                ident[:])
            drift_i = small.tile([128, 128], i32, tag="drifti")
            nc.vector.tensor_copy(out=drift_i[:ncols, :], in_=tpsum[:ncols, :])
            nc.sync.dma_start(
                drift_d.rearrange("(a b) -> a b", b=128),
                drift_i[:ncols, :],
            )

    nc.compile()
    return nc


def prep_inputs(x, noise, W1, b1, W2, b2, centroid, dis_median, mad,
                n_tiles=BS // TOK_TILE, n_cores=N_CORES):
    """Host-side preparation of per-core input maps."""
    bs = n_tiles * TOK_TILE
    x = np.asarray(x, dtype=np.float32)
    noise = np.asarray(noise, dtype=np.float32)
    W1 = np.asarray(W1, dtype=np.float32)
    b1 = np.asarray(b1, dtype=np.float32)
    W2 = np.asarray(W2, dtype=np.float32)
    b2 = np.asarray(b2, dtype=np.float32)
    centroid = np.asarray(centroid, dtype=np.float32)
    dis_median = np.asarray(dis_median, dtype=np.float32)
    mad = np.asarray(mad, dtype=np.float32)

    xn = x + noise

    W1s = np.ascontiguousarray(
        W1.reshape(KC1, 128, H).transpose(1, 0, 2)).astype(BF16)
    W2s = np.ascontiguousarray(
        W2.reshape(FC1, 128, D_LAT).transpose(1, 0, 2)).astype(BF16)
    b1s = np.ascontiguousarray(b1.reshape(FC1, 128).T)
    cTs = np.ascontiguousarray(centroid.T).astype(BF16)

    c2 = (centroid * centroid).sum(1)
    pre_f = PRE_SHIFT - 0.5 * c2 + centroid @ b2
    pre_hi = pre_f.astype(BF16)
    pre_lo = (pre_f - pre_hi.astype(np.float32)).astype(BF16)
    pre = np.ascontiguousarray(
        np.stack([pre_hi, pre_lo])[None, :, :])            # [1, 2, K]

    hi = dis_median + MAD_THRESHOLD * mad
    lo = dis_median - MAD_THRESHOLD * mad
    A = (hi * hi).astype(np.float32)
    Bv = np.where(lo > 0, lo * lo, -1.0).astype(np.float32)
    ctab = np.zeros((K, TAB_W), dtype=np.float32)
    ctab[:, :D_LAT] = centroid - b2[None, :]
    ctab[:, 128] = A
    ctab[:, 129] = Bv

    def shard_T(a, core):
        s = a[core * bs:(core + 1) * bs].astype(BF16)       # [bs, 512]
        sT = s.T                                            # [512, bs]
        blk = sT.reshape(KC1, 128, n_tiles, TOK_TILE).transpose(2, 0, 1, 3)
        return np.ascontiguousarray(blk)

    in_maps = []
    for core in range(n_cores):
        in_maps.append({
            "xT": shard_T(x, core),
            "xnT": shard_T(xn, core),
            "W1s": W1s,
            "W2s": W2s,
            "b1s": b1s,
            "cTs": cTs,
            "pre": pre,
            "ctab": ctab,
        })
    return in_maps


def kernel(x, noise, W1, b1, W2, b2, centroid, dis_median, mad):
    from concourse.bass_utils import run_bass_kernel_spmd

    nc = build_program()
    in_maps = prep_inputs(x, noise, W1, b1, W2, b2, centroid,
                          dis_median, mad)
    res = run_bass_kernel_spmd(nc, in_maps, core_ids=list(range(N_CORES)))
    out = np.concatenate([r["drift"] for r in res.results])
    return out.astype(np.int32)


# revision 13
# speedup vs baseline: 54.3498x; 54.3498x over previous
"""Trainium2 Bass kernel for nn_Detector (retrieval_knn drift detector).

Pipeline (per token):
    z1 = relu(x @ W1 + b1) @ W2 + b2
    cls = argmin_j ||z1 - centroid_j||
    z2 = relu((x+noise) @ W1 + b1) @ W2 + b2
    dis = ||z2 - centroid_cls||
    drift = |dis - med_cls| / mad_cls > 3.5

Strategy: pure data-parallel over 8 NeuronCores (8192 tokens each).
On-chip activations are feature-major ([feat, tok]) so the contraction dim
sits on partitions.  All matmuls in bf16 (output drift bits have >4.5 sigma
margin vs the 3.5 threshold, verified against the fp32 reference).

Algebraic rewrites baked in on the host:
  - argmin_j ||z1-c_j||^2 == argmax_j (z1.c_j - 0.5||c_j||^2).  The per-j
    constant (300 - 0.5||c_j||^2 + b2.c_j) is added via a rank-1 matmul
    preload into PSUM (ones (x) pre_j), split hi/lo bf16 for precision,
    which also folds away the b2 bias of the first encoder pass.
  - drift = (d2 > A_cls) | (d2 < B_cls) with d2 = ||z2' - (c_cls - b2)||^2,
    z2' the bias-free second encoding, A = (med+3.5*mad)^2 and
    B = (med-3.5*mad)^2 if med > 3.5*mad else -1.  No sqrt, no division,
    no med/mad gathers.
  - gather table rows [c_j - b2 (128 f32), A_j, B_j, pad, pad] fetched by
    one indirect DMA per 512-token tile.
"""

import numpy as np
import ml_dtypes

import concourse.bass as bass
import concourse.bacc as bacc
import concourse.mybir as mybir
import concourse.tile as tile
from concourse.masks import make_identity

BF16 = ml_dtypes.bfloat16

B, D_IN, H, D_LAT, K = 65536, 512, 256, 128, 1000
MAD_THRESHOLD = 3.5
N_CORES = 8
BS = B // N_CORES            # tokens per core
TOK_TILE = 512               # tokens per pipeline tile
KC1 = D_IN // 128            # 4  K-chunks for layer 1
FC1 = H // 128               # 2  feature chunks of the hidden layer
TAB_W = 132                  # gather-table row width (128 + A + B + 2 pad)
PRE_SHIFT = 0.0              # no offset: |pre| stays small so the single
                             # bf16 rank-1 preload keeps ~0.1 precision


def build_program(n_tiles=BS // TOK_TILE, enable_asserts=False,
                  debug_taps=False):
    """Build the per-core Bass program.  Returns (nc, names) where names maps
    logical tensors to dram tensor names."""
    bs = n_tiles * TOK_TILE
    nc = bacc.Bacc(
        "TRN2",
        target_bir_lowering=False,
        debug=False,
        enable_asserts=enable_asserts,
        num_devices=N_CORES,
    )
    f32, bf16, i32, u32 = (
        mybir.dt.float32, mybir.dt.bfloat16, mybir.dt.int32, mybir.dt.uint32,
    )

    xT = nc.dram_tensor("xT", [n_tiles, KC1, 128, TOK_TILE], bf16,
                        kind="ExternalInput").ap()
    xnT = nc.dram_tensor("xnT", [n_tiles, KC1, 128, TOK_TILE], bf16,
                         kind="ExternalInput").ap()
    W1s_d = nc.dram_tensor("W1s", [128, KC1, H], bf16, kind="ExternalInput").ap()
    W2s_d = nc.dram_tensor("W2s", [128, FC1, D_LAT], bf16,
                           kind="ExternalInput").ap()
    b1s_d = nc.dram_tensor("b1s", [128, FC1], f32, kind="ExternalInput").ap()
    cTs_d = nc.dram_tensor("cTs", [128, K], bf16, kind="ExternalInput").ap()
    pre_d = nc.dram_tensor("pre", [1, 2, K], bf16, kind="ExternalInput").ap()
    ctab = nc.dram_tensor("ctab", [K, TAB_W], f32, kind="ExternalInput").ap()
    drift_d = nc.dram_tensor("drift", [bs], i32, kind="ExternalOutput").ap()

    CH = TOK_TILE // 128     # 4 token chunks per tile
    if debug_taps:
        cls_dbg = nc.dram_tensor("cls_dbg", [n_tiles, 128, CH], u32,
                                 kind="ExternalOutput").ap()
        d2_dbg = nc.dram_tensor("d2_dbg", [n_tiles, 128, CH], f32,
                                kind="ExternalOutput").ap()
        m8_dbg = nc.dram_tensor("m8_dbg", [n_tiles, 128, CH, 8], f32,
                                kind="ExternalOutput").ap()
        tab_dbg = nc.dram_tensor("tab_dbg", [n_tiles, 128, CH, TAB_W], f32,
                                 kind="ExternalOutput").ap()

    with tile.TileContext(nc) as tc:
        with (
            tc.tile_pool(name="const", bufs=1) as const,
            tc.tile_pool(name="xin", bufs=16) as xin,
            tc.tile_pool(name="hsb", bufs=8) as hsb,
            tc.tile_pool(name="zsb", bufs=2) as zsb,
            tc.tile_pool(name="small", bufs=4) as small,
            tc.tile_pool(name="tab", bufs=2) as tabp,
            tc.tile_pool(name="acc", bufs=1) as accp,
            tc.tile_pool(name="mm", bufs=3, space="PSUM") as mmp,
            tc.tile_pool(name="gp", bufs=2, space="PSUM") as gpp,
            tc.tile_pool(name="z2r", bufs=1, space="PSUM") as z2rp,
        ):
            # ---- constants -------------------------------------------------
            W1s = const.tile([128, KC1, H], bf16)
            nc.sync.dma_start(W1s[:], W1s_d[:])
            W2s = const.tile([128, FC1, D_LAT], bf16)
            nc.sync.dma_start(W2s[:], W2s_d[:])
            b1s = const.tile([128, FC1], f32)
            nc.sync.dma_start(b1s[:], b1s_d[:])
            cTs = const.tile([128, K], bf16)
            nc.sync.dma_start(cTs[:], cTs_d[:])
            pre = const.tile([1, 2, K], bf16)
            nc.sync.dma_start(pre[:], pre_d[:])
            ones1 = const.tile([1, 128], bf16)
            nc.gpsimd.memset(ones1[:], 1.0)
            ident = const.tile([128, 128], f32)
            make_identity(nc, ident[:])

            driftacc = accp.tile([128, n_tiles * CH], f32)

            # G matmul N-halves (<=512 free dim per PSUM bank)
            halves = [(0, 512), (512, K)]

            for i in range(n_tiles):
                # ---- load inputs (feature-major bf16) ----------------------
                xts = []
                xnts = []
                for kc in range(KC1):
                    t = xin.tile([128, TOK_TILE], bf16, tag="xin")
                    nc.sync.dma_start(t[:], xT[i, kc])
                    xts.append(t)
                for kc in range(KC1):
                    t = xin.tile([128, TOK_TILE], bf16, tag="xin")
                    nc.sync.dma_start(t[:], xnT[i, kc])
                    xnts.append(t)

                # ---- layer 1 (both passes) + relu --------------------------
                h1b, h2b = [], []
                for src, dst in ((xts, h1b), (xnts, h2b)):
                    for fc in range(FC1):
                        hT = mmp.tile([128, TOK_TILE], mybir.dt.float32,
                                      tag="mm")
                        for kc in range(KC1):
                            nc.tensor.matmul(
                                hT[:],
                                lhsT=W1s[:, kc, fc * 128:(fc + 1) * 128],
                                rhs=src[kc][:],
                                start=(kc == 0),
                                stop=(kc == KC1 - 1),
                            )
                        hb = hsb.tile([128, TOK_TILE], bf16, tag="h")
                        nc.scalar.activation(
                            hb[:], hT[:], mybir.ActivationFunctionType.Relu,
                            bias=b1s[:, fc:fc + 1],
                        )
                        dst.append(hb)

                # ---- layer 2, clean pass (feature-major, bias folded) ------
                zT = mmp.tile([128, TOK_TILE], mybir.dt.float32, tag="mm")
                for kc in range(FC1):
                    nc.tensor.matmul(
                        zT[:], lhsT=W2s[:, kc, :], rhs=h1b[kc][:],
                        start=(kc == 0), stop=(kc == FC1 - 1),
                    )
                z1b = zsb.tile([128, TOK_TILE], bf16, tag="z1")
                nc.scalar.activation(z1b[:], zT[:],
                                     mybir.ActivationFunctionType.Copy)

                cls4 = small.tile([128, CH, 8], u32, tag="cls")
                d2c = small.tile([128, CH], mybir.dt.float32, tag="d2")

                for c in range(CH):
                    csl = slice(c * 128, (c + 1) * 128)

                    # ---- scores G = z1.c_j + pre_j  (PSUM, fp32) -----------
                    G = gpp.tile([128, 1024], mybir.dt.float32, tag="G")
                    for lo, hi in halves:
                        nc.tensor.matmul(
                            G[:, lo:lo + (hi - lo)],
                            lhsT=ones1[:], rhs=pre[:, 0, lo:hi],
                            start=True, stop=False,
                        )
                        nc.tensor.matmul(
                            G[:, lo:lo + (hi - lo)],
                            lhsT=z1b[:, csl], rhs=cTs[:, lo:hi],
                            start=False, stop=True,
                        )

                    # ---- argmax over centroids -----------------------------
                    m8 = small.tile([128, 8], mybir.dt.float32, tag="m8")
                    nc.vector.max(out=m8[:], in_=G[:, :K])
                    nc.vector.max_index(
                        out=cls4[:, c, :], in_max=m8[:], in_values=G[:, :K],
                    )
                    if debug_taps:
                        nc.sync.dma_start(m8_dbg[i, :, c, :], m8[:])

                # ---- gather [c_j - b2, A, B, pad] rows by cls --------------
                # One indirect DMA per 128-token chunk with [128, 1] offsets:
                # multi-offset-per-partition gathers pair offsets to output
                # rows in a different order on HW than in the simulator.
                clsc = small.tile([128, CH], u32, tag="clsc")
                nc.vector.tensor_scalar(
                    out=clsc[:], in0=cls4[:, :, 0],
                    scalar1=K - 1, scalar2=None, op0=mybir.AluOpType.min,
                )
                tabsel = tabp.tile([128, CH, TAB_W], mybir.dt.float32,
                                   tag="tab")
                for c in range(CH):
                    nc.gpsimd.indirect_dma_start(
                        out=tabsel[:, c, :],
                        out_offset=None,
                        in_=ctab[:],
                        in_offset=bass.IndirectOffsetOnAxis(
                            ap=clsc[:, c:c + 1], axis=0),
                    )

                for c in range(CH):
                    csl = slice(c * 128, (c + 1) * 128)
                    z2r = z2rp.tile([128, D_LAT], mybir.dt.float32, tag="z2r")
                    for kc in range(FC1):
                        nc.tensor.matmul(
                            z2r[:], lhsT=h2b[kc][:, csl], rhs=W2s[:, kc, :],
                            start=(kc == 0), stop=(kc == FC1 - 1),
                        )
                    # diff = z2r - csel   (bf16)
                    diff = small.tile([128, D_LAT], bf16, tag="diff")
                    nc.vector.scalar_tensor_tensor(
                        out=diff[:],
                        in0=tabsel[:, c, 0:D_LAT],
                        scalar=-1.0,
                        in1=z2r[:],
                        op0=mybir.AluOpType.mult,
                        op1=mybir.AluOpType.add,
                    )
                    # d2 = sum(diff*diff)   (ScalarE: Square with accumulate)
                    junk = small.tile([128, D_LAT], bf16, tag="junk")
                    nc.scalar.activation(
                        junk[:], diff[:],
                        mybir.ActivationFunctionType.Square,
                        accum_out=d2c[:, c:c + 1],
                    )

                # ---- drift = (d2 > A) | (d2 < B) ---------------------------
                ga = small.tile([128, CH], mybir.dt.float32, tag="ga")
                gb = small.tile([128, CH], mybir.dt.float32, tag="gb")
                nc.vector.tensor_tensor(
                    out=ga[:], in0=d2c[:], in1=tabsel[:, :, 128],
                    op=mybir.AluOpType.is_gt,
                )
                nc.vector.tensor_tensor(
                    out=gb[:], in0=d2c[:], in1=tabsel[:, :, 129],
                    op=mybir.AluOpType.is_lt,
                )
                nc.vector.tensor_tensor(
                    out=driftacc[:, i * CH:(i + 1) * CH],
                    in0=ga[:], in1=gb[:], op=mybir.AluOpType.max,
                )
                if debug_taps:
                    nc.sync.dma_start(cls_dbg[i], clsc[:])
                    nc.sync.dma_start(d2_dbg[i], d2c[:])
                    nc.sync.dma_start(tab_dbg[i], tabsel[:])

            # ---- transpose [128, n_tiles*CH] -> token order and store ------
            ncols = n_tiles * CH
            tpsum = z2rp.tile([128, 128], mybir.dt.float32, tag="z2r")
            nc.tensor.transpose(tpsum[:ncols, :], driftacc[:, :ncols],
                                ident[:])
            drift_i = small.tile([128, 128], i32, tag="drifti")
            nc.vector.tensor_copy(out=drift_i[:ncols, :], in_=tpsum[:ncols, :])
            nc.sync.dma_start(
                drift_d.rearrange("(a b) -> a b", b=128),
                drift_i[:ncols, :],
            )

    nc.compile()
    return nc


def prep_inputs(x, noise, W1, b1, W2, b2, centroid, dis_median, mad,
                n_tiles=BS // TOK_TILE, n_cores=N_CORES):
    """Host-side preparation of per-core input maps."""
    bs = n_tiles * TOK_TILE
    x = np.asarray(x, dtype=np.float32)
    noise = np.asarray(noise, dtype=np.float32)
    W1 = np.asarray(W1, dtype=np.float32)
    b1 = np.asarray(b1, dtype=np.float32)
    W2 = np.asarray(W2, dtype=np.float32)
    b2 = np.asarray(b2, dtype=np.float32)
    centroid = np.asarray(centroid, dtype=np.float32)
    dis_median = np.asarray(dis_median, dtype=np.float32)
    mad = np.asarray(mad, dtype=np.float32)

    xn = x + noise

    W1s = np.ascontiguousarray(
        W1.reshape(KC1, 128, H).transpose(1, 0, 2)).astype(BF16)
    W2s = np.ascontiguousarray(
        W2.reshape(FC1, 128, D_LAT).transpose(1, 0, 2)).astype(BF16)
    b1s = np.ascontiguousarray(b1.reshape(FC1, 128).T)
    cTs = np.ascontiguousarray(centroid.T).astype(BF16)

    c2 = (centroid * centroid).sum(1)
    pre_f = PRE_SHIFT - 0.5 * c2 + centroid @ b2
    pre_hi = pre_f.astype(BF16)
    pre_lo = (pre_f - pre_hi.astype(np.float32)).astype(BF16)
    pre = np.ascontiguousarray(
        np.stack([pre_hi, pre_lo])[None, :, :])            # [1, 2, K]

    hi = dis_median + MAD_THRESHOLD * mad
    lo = dis_median - MAD_THRESHOLD * mad
    A = (hi * hi).astype(np.float32)
    Bv = np.where(lo > 0, lo * lo, -1.0).astype(np.float32)
    ctab = np.zeros((K, TAB_W), dtype=np.float32)
    ctab[:, :D_LAT] = centroid - b2[None, :]
    ctab[:, 128] = A
    ctab[:, 129] = Bv

    def shard_T(a, core):
        s = a[core * bs:(core + 1) * bs].astype(BF16)       # [bs, 512]
        sT = s.T                                            # [512, bs]
        blk = sT.reshape(KC1, 128, n_tiles, TOK_TILE).transpose(2, 0, 1, 3)
        return np.ascontiguousarray(blk)

    in_maps = []
    for core in range(n_cores):
        in_maps.append({
            "xT": shard_T(x, core),
            "xnT": shard_T(xn, core),
            "W1s": W1s,
            "W2s": W2s,
            "b1s": b1s,
            "cTs": cTs,
            "pre": pre,
            "ctab": ctab,
        })
    return in_maps


def kernel(x, noise, W1, b1, W2, b2, centroid, dis_median, mad):
    from concourse.bass_utils import run_bass_kernel_spmd

    nc = build_program()
    in_maps = prep_inputs(x, noise, W1, b1, W2, b2, centroid,
                          dis_median, mad)
    res = run_bass_kernel_spmd(nc, in_maps, core_ids=list(range(N_CORES)))
    out = np.concatenate([r["drift"] for r in res.results])
    return out.astype(np.int32)


# revision 19
# speedup vs baseline: 299.9460x; 5.5188x over previous
"""Trainium2 Bass kernel for nn_Detector (retrieval_knn drift detector).

Pipeline (per token):
    z1 = relu(x @ W1 + b1) @ W2 + b2
    cls = argmin_j ||z1 - centroid_j||
    z2 = relu((x+noise) @ W1 + b1) @ W2 + b2
    dis = ||z2 - centroid_cls||
    drift = |dis - med_cls| / mad_cls > 3.5

Strategy: pure data-parallel over 8 NeuronCores (8192 tokens each).
On-chip activations are feature-major ([feat, tok]) so the contraction dim
sits on partitions.  All matmuls in bf16 (output drift bits have >4.5 sigma
margin vs the 3.5 threshold, verified against the fp32 reference).

Algebraic rewrites baked in on the host:
  - argmin_j ||z1-c_j||^2 == argmax_j (z1.c_j - 0.5||c_j||^2).  The per-j
    constant (-0.5||c_j||^2 + b2.c_j) is added via a rank-1 matmul preload
    into PSUM (ones (x) pre_j, bf16), which also folds away the b2 bias of
    the first encoder pass.
  - drift = (d2 > A_cls) | (d2 < B_cls) with d2 = ||z2' - (c_cls - b2)||^2,
    z2' the bias-free second encoding, A = (med+3.5*mad)^2 and
    B = (med-3.5*mad)^2 if med > 3.5*mad else -1.  No sqrt, no division,
    no med/mad gathers.
  - gather table rows [c_j - b2 (128 f32), A_j, B_j, pad, pad] fetched by
    one indirect DMA per 512-token tile.
"""

import numpy as np
import ml_dtypes

import concourse.bass as bass
import concourse.bacc as bacc
import concourse.mybir as mybir
import concourse.tile as tile
from concourse.masks import make_identity

BF16 = ml_dtypes.bfloat16

B, D_IN, H, D_LAT, K = 65536, 512, 256, 128, 1000
MAD_THRESHOLD = 3.5
N_CORES = 8
BS = B // N_CORES            # tokens per core
TOK_TILE = 512               # tokens per pipeline tile
KC1 = D_IN // 128            # 4  K-chunks for layer 1
FC1 = H // 128               # 2  feature chunks of the hidden layer
TAB_W = 132                  # gather-table row width (128 + A + B + 2 pad)
PRE_SHIFT = 0.0              # no offset: |pre| stays small so the single
                             # bf16 rank-1 preload keeps ~0.1 precision


def build_program(n_tiles=BS // TOK_TILE, enable_asserts=False,
                  debug_taps=False):
    """Build the per-core Bass program.  Returns (nc, names) where names maps
    logical tensors to dram tensor names."""
    bs = n_tiles * TOK_TILE
    nc = bacc.Bacc(
        "TRN2",
        target_bir_lowering=False,
        debug=False,
        enable_asserts=enable_asserts,
        num_devices=N_CORES,
    )
    f32, bf16, i32, u32 = (
        mybir.dt.float32, mybir.dt.bfloat16, mybir.dt.int32, mybir.dt.uint32,
    )

    xT = nc.dram_tensor("xT", [n_tiles, KC1, 128, TOK_TILE], bf16,
                        kind="ExternalInput").ap()
    xnT = nc.dram_tensor("xnT", [n_tiles, KC1, 128, TOK_TILE], bf16,
                         kind="ExternalInput").ap()
    W1s_d = nc.dram_tensor("W1s", [128, KC1, H], bf16, kind="ExternalInput").ap()
    W2s_d = nc.dram_tensor("W2s", [128, FC1, D_LAT], bf16,
                           kind="ExternalInput").ap()
    b1s_d = nc.dram_tensor("b1s", [128, FC1], f32, kind="ExternalInput").ap()
    cTs_d = nc.dram_tensor("cTs", [128, K], bf16, kind="ExternalInput").ap()
    pre_d = nc.dram_tensor("pre", [1, 2, K], bf16, kind="ExternalInput").ap()
    ctab = nc.dram_tensor("ctab", [K, TAB_W], f32, kind="ExternalInput").ap()
    drift_d = nc.dram_tensor("drift", [bs], i32, kind="ExternalOutput").ap()

    CH = TOK_TILE // 128     # 4 token chunks per tile
    if debug_taps:
        cls_dbg = nc.dram_tensor("cls_dbg", [n_tiles, 128, CH], u32,
                                 kind="ExternalOutput").ap()
        d2_dbg = nc.dram_tensor("d2_dbg", [n_tiles, 128, CH], f32,
                                kind="ExternalOutput").ap()
        m8_dbg = nc.dram_tensor("m8_dbg", [n_tiles, 128, CH, 8], f32,
                                kind="ExternalOutput").ap()
        tab_dbg = nc.dram_tensor("tab_dbg", [n_tiles, 128, CH, TAB_W], f32,
                                 kind="ExternalOutput").ap()

    with tile.TileContext(nc) as tc:
        with (
            tc.tile_pool(name="const", bufs=1) as const,
            tc.tile_pool(name="xin", bufs=16) as xin,
            tc.tile_pool(name="hsb", bufs=8) as hsb,
            tc.tile_pool(name="zsb", bufs=2) as zsb,
            tc.tile_pool(name="small", bufs=4) as small,
            tc.tile_pool(name="tab", bufs=2) as tabp,
            tc.tile_pool(name="acc", bufs=1) as accp,
            tc.tile_pool(name="mm", bufs=3, space="PSUM") as mmp,
            tc.tile_pool(name="gp", bufs=2, space="PSUM") as gpp,
            tc.tile_pool(name="z2r", bufs=1, space="PSUM") as z2rp,
        ):
            # ---- constants -------------------------------------------------
            W1s = const.tile([128, KC1, H], bf16)
            nc.sync.dma_start(W1s[:], W1s_d[:])
            W2s = const.tile([128, FC1, D_LAT], bf16)
            nc.sync.dma_start(W2s[:], W2s_d[:])
            b1s = const.tile([128, FC1], f32)
            nc.sync.dma_start(b1s[:], b1s_d[:])
            cTs = const.tile([128, K], bf16)
            nc.sync.dma_start(cTs[:], cTs_d[:])
            pre = const.tile([1, 2, K], bf16)
            nc.sync.dma_start(pre[:], pre_d[:])
            ones1 = const.tile([1, 128], bf16)
            nc.gpsimd.memset(ones1[:], 1.0)
            ident = const.tile([128, 128], f32)
            make_identity(nc, ident[:])

            driftacc = accp.tile([128, n_tiles * CH], f32)

            # G matmul N-halves (<=512 free dim per PSUM bank)
            halves = [(0, 512), (512, K)]

            for i in range(n_tiles):
                # ---- load inputs (feature-major bf16) ----------------------
                xts = []
                xnts = []
                for kc in range(KC1):
                    t = xin.tile([128, TOK_TILE], bf16, tag="xin")
                    nc.sync.dma_start(t[:], xT[i, kc])
                    xts.append(t)
                for kc in range(KC1):
                    t = xin.tile([128, TOK_TILE], bf16, tag="xin")
                    nc.sync.dma_start(t[:], xnT[i, kc])
                    xnts.append(t)

                # ---- layer 1 (both passes) + relu --------------------------
                h1b, h2b = [], []
                for src, dst in ((xts, h1b), (xnts, h2b)):
                    for fc in range(FC1):
                        hT = mmp.tile([128, TOK_TILE], mybir.dt.float32,
                                      tag="mm")
                        for kc in range(KC1):
                            nc.tensor.matmul(
                                hT[:],
                                lhsT=W1s[:, kc, fc * 128:(fc + 1) * 128],
                                rhs=src[kc][:],
                                start=(kc == 0),
                                stop=(kc == KC1 - 1),
                            )
                        hb = hsb.tile([128, TOK_TILE], bf16, tag="h")
                        nc.scalar.activation(
                            hb[:], hT[:], mybir.ActivationFunctionType.Relu,
                            bias=b1s[:, fc:fc + 1],
                        )
                        dst.append(hb)

                # ---- layer 2, clean pass (feature-major, bias folded) ------
                zT = mmp.tile([128, TOK_TILE], mybir.dt.float32, tag="mm")
                for kc in range(FC1):
                    nc.tensor.matmul(
                        zT[:], lhsT=W2s[:, kc, :], rhs=h1b[kc][:],
                        start=(kc == 0), stop=(kc == FC1 - 1),
                    )
                z1b = zsb.tile([128, TOK_TILE], bf16, tag="z1")
                nc.scalar.activation(z1b[:], zT[:],
                                     mybir.ActivationFunctionType.Copy)

                cls4 = small.tile([128, CH, 8], u32, tag="cls")
                d2c = small.tile([128, CH], mybir.dt.float32, tag="d2")

                for c in range(CH):
                    csl = slice(c * 128, (c + 1) * 128)

                    # ---- scores G = z1.c_j + pre_j  (PSUM, fp32) -----------
                    G = gpp.tile([128, 1024], mybir.dt.float32, tag="G")
                    for lo, hi in halves:
                        nc.tensor.matmul(
                            G[:, lo:lo + (hi - lo)],
                            lhsT=ones1[:], rhs=pre[:, 0, lo:hi],
                            start=True, stop=False,
                        )
                        nc.tensor.matmul(
                            G[:, lo:lo + (hi - lo)],
                            lhsT=z1b[:, csl], rhs=cTs[:, lo:hi],
                            start=False, stop=True,
                        )

                    # ---- argmax over centroids -----------------------------
                    m8 = small.tile([128, 8], mybir.dt.float32, tag="m8")
                    nc.vector.max(out=m8[:], in_=G[:, :K])
                    nc.vector.max_index(
                        out=cls4[:, c, :], in_max=m8[:], in_values=G[:, :K],
                    )
                    if debug_taps:
                        nc.sync.dma_start(m8_dbg[i, :, c, :], m8[:])

                # ---- gather [c_j - b2, A, B, pad] rows by cls --------------
                # One indirect DMA per 128-token chunk with [128, 1] offsets:
                # multi-offset-per-partition gathers pair offsets to output
                # rows in a different order on HW than in the simulator.
                clsc = small.tile([128, CH], u32, tag="clsc")
                nc.vector.tensor_scalar(
                    out=clsc[:], in0=cls4[:, :, 0],
                    scalar1=K - 1, scalar2=None, op0=mybir.AluOpType.min,
                )
                tabsel = tabp.tile([128, CH, TAB_W], mybir.dt.float32,
                                   tag="tab")
                for c in range(CH):
                    nc.gpsimd.indirect_dma_start(
                        out=tabsel[:, c, :],
                        out_offset=None,
                        in_=ctab[:],
                        in_offset=bass.IndirectOffsetOnAxis(
                            ap=clsc[:, c:c + 1], axis=0),
                    )

                for c in range(CH):
                    csl = slice(c * 128, (c + 1) * 128)
                    z2r = z2rp.tile([128, D_LAT], mybir.dt.float32, tag="z2r")
                    for kc in range(FC1):
                        nc.tensor.matmul(
                            z2r[:], lhsT=h2b[kc][:, csl], rhs=W2s[:, kc, :],
                            start=(kc == 0), stop=(kc == FC1 - 1),
                        )
                    # diff = z2r - csel   (bf16)
                    diff = small.tile([128, D_LAT], bf16, tag="diff")
                    nc.vector.scalar_tensor_tensor(
                        out=diff[:],
                        in0=tabsel[:, c, 0:D_LAT],
                        scalar=-1.0,
                        in1=z2r[:],
                        op0=mybir.AluOpType.mult,
                        op1=mybir.AluOpType.add,
                    )
                    # d2 = sum(diff*diff)   (ScalarE: Square with accumulate)
                    junk = small.tile([128, D_LAT], bf16, tag="junk")
                    nc.scalar.activation(
                        junk[:], diff[:],
                        mybir.ActivationFunctionType.Square,
                        accum_out=d2c[:, c:c + 1],
                    )

                # ---- drift = (d2 > A) | (d2 < B) ---------------------------
                ga = small.tile([128, CH], mybir.dt.float32, tag="ga")
                gb = small.tile([128, CH], mybir.dt.float32, tag="gb")
                nc.vector.tensor_tensor(
                    out=ga[:], in0=d2c[:], in1=tabsel[:, :, 128],
                    op=mybir.AluOpType.is_gt,
                )
                nc.vector.tensor_tensor(
                    out=gb[:], in0=d2c[:], in1=tabsel[:, :, 129],
                    op=mybir.AluOpType.is_lt,
                )
                nc.vector.tensor_tensor(
                    out=driftacc[:, i * CH:(i + 1) * CH],
                    in0=ga[:], in1=gb[:], op=mybir.AluOpType.max,
                )
                if debug_taps:
                    nc.sync.dma_start(cls_dbg[i], clsc[:])
                    nc.sync.dma_start(d2_dbg[i], d2c[:])
                    nc.sync.dma_start(tab_dbg[i], tabsel[:])

            # ---- transpose [128, n_tiles*CH] -> token order and store ------
            ncols = n_tiles * CH
            tpsum = z2rp.tile([128, 128], mybir.dt.float32, tag="z2r")
            nc.tensor.transpose(tpsum[:ncols, :], driftacc[:, :ncols],
                                ident[:])
            drift_i = small.tile([128, 128], i32, tag="drifti")
            nc.vector.tensor_copy(out=drift_i[:ncols, :], in_=tpsum[:ncols, :])
            nc.sync.dma_start(
                drift_d.rearrange("(a b) -> a b", b=128),
                drift_i[:ncols, :],
            )

    nc.compile()
    return nc


def prep_inputs(x, noise, W1, b1, W2, b2, centroid, dis_median, mad,
                n_tiles=BS // TOK_TILE, n_cores=N_CORES):
    """Host-side preparation of per-core input maps."""
    bs = n_tiles * TOK_TILE
    x = np.asarray(x, dtype=np.float32)
    noise = np.asarray(noise, dtype=np.float32)
    W1 = np.asarray(W1, dtype=np.float32)
    b1 = np.asarray(b1, dtype=np.float32)
    W2 = np.asarray(W2, dtype=np.float32)
    b2 = np.asarray(b2, dtype=np.float32)
    centroid = np.asarray(centroid, dtype=np.float32)
    dis_median = np.asarray(dis_median, dtype=np.float32)
    mad = np.asarray(mad, dtype=np.float32)

    xn = x + noise

    W1s = np.ascontiguousarray(
        W1.reshape(KC1, 128, H).transpose(1, 0, 2)).astype(BF16)
    W2s = np.ascontiguousarray(
        W2.reshape(FC1, 128, D_LAT).transpose(1, 0, 2)).astype(BF16)
    b1s = np.ascontiguousarray(b1.reshape(FC1, 128).T)
    cTs = np.ascontiguousarray(centroid.T).astype(BF16)

    c2 = (centroid * centroid).sum(1)
    pre_f = PRE_SHIFT - 0.5 * c2 + centroid @ b2
    pre_hi = pre_f.astype(BF16)
    pre_lo = (pre_f - pre_hi.astype(np.float32)).astype(BF16)
    pre = np.ascontiguousarray(
        np.stack([pre_hi, pre_lo])[None, :, :])            # [1, 2, K]

    hi = dis_median + MAD_THRESHOLD * mad
    lo = dis_median - MAD_THRESHOLD * mad
    A = (hi * hi).astype(np.float32)
    Bv = np.where(lo > 0, lo * lo, -1.0).astype(np.float32)
    ctab = np.zeros((K, TAB_W), dtype=np.float32)
    ctab[:, :D_LAT] = centroid - b2[None, :]
    ctab[:, 128] = A
    ctab[:, 129] = Bv

    def shard_T(a, core):
        s = a[core * bs:(core + 1) * bs].astype(BF16)       # [bs, 512]
        sT = s.T                                            # [512, bs]
        blk = sT.reshape(KC1, 128, n_tiles, TOK_TILE).transpose(2, 0, 1, 3)
        return np.ascontiguousarray(blk)

    in_maps = []
    for core in range(n_cores):
        in_maps.append({
            "xT": shard_T(x, core),
            "xnT": shard_T(xn, core),
            "W1s": W1s,
            "W2s": W2s,
            "b1s": b1s,
            "cTs": cTs,
            "pre": pre,
            "ctab": ctab,
        })
    return in_maps


_BUILD_CACHE = {}


def kernel(x, noise, W1, b1, W2, b2, centroid, dis_median, mad):
    from concourse.bass_utils import run_bass_kernel_spmd

    nc = _BUILD_CACHE.get("nc")
    if nc is None:
        nc = _BUILD_CACHE["nc"] = build_program()
    in_maps = prep_inputs(x, noise, W1, b1, W2, b2, centroid,
                          dis_median, mad)
    res = run_bass_kernel_spmd(nc, in_maps, core_ids=list(range(N_CORES)))
    out = np.concatenate([r["drift"] for r in res.results])
    return out.astype(np.int32)


# revision 22
# speedup vs baseline: 306.8263x; 1.0229x over previous
"""Trainium2 Bass kernel for nn_Detector (retrieval_knn drift detector).

Pipeline (per token):
    z1 = relu(x @ W1 + b1) @ W2 + b2
    cls = argmin_j ||z1 - centroid_j||
    z2 = relu((x+noise) @ W1 + b1) @ W2 + b2
    dis = ||z2 - centroid_cls||
    drift = |dis - med_cls| / mad_cls > 3.5

Strategy: pure data-parallel over 8 NeuronCores (8192 tokens each).
On-chip activations are feature-major ([feat, tok]) so the contraction dim
sits on partitions.  All matmuls in bf16 (output drift bits have >4.5 sigma
margin vs the 3.5 threshold, verified against the fp32 reference).

Algebraic rewrites baked in on the host:
  - argmin_j ||z1-c_j||^2 == argmax_j (z1.c_j - 0.5||c_j||^2).  The per-j
    constant (-0.5||c_j||^2 + b2.c_j) is added via a rank-1 matmul preload
    into PSUM (ones (x) pre_j, bf16), which also folds away the b2 bias of
    the first encoder pass.
  - drift = (d2 > A_cls) | (d2 < B_cls) with d2 = ||z2' - (c_cls - b2)||^2,
    z2' the bias-free second encoding, A = (med+3.5*mad)^2 and
    B = (med-3.5*mad)^2 if med > 3.5*mad else -1.  No sqrt, no division,
    no med/mad gathers.
  - gather table rows [c_j - b2 (128 f32), A_j, B_j, pad, pad] fetched by
    one indirect DMA per 512-token tile.
"""

import numpy as np
import ml_dtypes

import concourse.bass as bass
import concourse.bacc as bacc
import concourse.mybir as mybir
import concourse.tile as tile
from concourse.masks import make_identity

BF16 = ml_dtypes.bfloat16

B, D_IN, H, D_LAT, K = 65536, 512, 256, 128, 1000
MAD_THRESHOLD = 3.5
N_CORES = 8
BS = B // N_CORES            # tokens per core
TOK_TILE = 512               # tokens per pipeline tile
KC1 = D_IN // 128            # 4  K-chunks for layer 1
FC1 = H // 128               # 2  feature chunks of the hidden layer
TAB_W = 132                  # gather-table row width (128 + A + B + 2 pad)
PRE_SHIFT = 0.0              # no offset: |pre| stays small so the single
                             # bf16 rank-1 preload keeps ~0.1 precision


def build_program(n_tiles=BS // TOK_TILE, enable_asserts=False,
                  debug_taps=False):
    """Build the per-core Bass program.  Returns (nc, names) where names maps
    logical tensors to dram tensor names."""
    bs = n_tiles * TOK_TILE
    nc = bacc.Bacc(
        "TRN2",
        target_bir_lowering=False,
        debug=False,
        enable_asserts=enable_asserts,
        num_devices=N_CORES,
    )
    f32, bf16, i32, u32 = (
        mybir.dt.float32, mybir.dt.bfloat16, mybir.dt.int32, mybir.dt.uint32,
    )

    xT = nc.dram_tensor("xT", [n_tiles, KC1, 128, TOK_TILE], bf16,
                        kind="ExternalInput").ap()
    xnT = nc.dram_tensor("xnT", [n_tiles, KC1, 128, TOK_TILE], bf16,
                         kind="ExternalInput").ap()
    W1s_d = nc.dram_tensor("W1s", [128, KC1, H], bf16, kind="ExternalInput").ap()
    W2s_d = nc.dram_tensor("W2s", [128, FC1, D_LAT], bf16,
                           kind="ExternalInput").ap()
    b1s_d = nc.dram_tensor("b1s", [128, FC1], f32, kind="ExternalInput").ap()
    cTs_d = nc.dram_tensor("cTs", [128, K], bf16, kind="ExternalInput").ap()
    pre_d = nc.dram_tensor("pre", [1, 2, K], bf16, kind="ExternalInput").ap()
    ctab = nc.dram_tensor("ctab", [K, TAB_W], f32, kind="ExternalInput").ap()
    drift_d = nc.dram_tensor("drift", [bs], i32, kind="ExternalOutput").ap()

    CH = TOK_TILE // 128     # 4 token chunks per tile
    if debug_taps:
        cls_dbg = nc.dram_tensor("cls_dbg", [n_tiles, 128, CH, 8], u32,
                                 kind="ExternalOutput").ap()
        d2_dbg = nc.dram_tensor("d2_dbg", [n_tiles, 128, CH], f32,
                                kind="ExternalOutput").ap()
        m8_dbg = nc.dram_tensor("m8_dbg", [n_tiles, 128, CH, 8], f32,
                                kind="ExternalOutput").ap()
        tab_dbg = nc.dram_tensor("tab_dbg", [n_tiles, 128, CH, TAB_W], f32,
                                 kind="ExternalOutput").ap()

    with tile.TileContext(nc) as tc:
        with (
            tc.tile_pool(name="const", bufs=1) as const,
            tc.tile_pool(name="xin", bufs=16) as xin,
            tc.tile_pool(name="hsb", bufs=8) as hsb,
            tc.tile_pool(name="zsb", bufs=2) as zsb,
            tc.tile_pool(name="small", bufs=4) as small,
            tc.tile_pool(name="tab", bufs=2) as tabp,
            tc.tile_pool(name="acc", bufs=1) as accp,
            tc.tile_pool(name="mm", bufs=3, space="PSUM") as mmp,
            tc.tile_pool(name="gp", bufs=2, space="PSUM") as gpp,
            tc.tile_pool(name="z2r", bufs=1, space="PSUM") as z2rp,
        ):
            # ---- constants -------------------------------------------------
            W1s = const.tile([128, KC1, H], bf16)
            nc.sync.dma_start(W1s[:], W1s_d[:])
            W2s = const.tile([128, FC1, D_LAT], bf16)
            nc.sync.dma_start(W2s[:], W2s_d[:])
            b1s = const.tile([128, FC1], f32)
            nc.sync.dma_start(b1s[:], b1s_d[:])
            cTs = const.tile([128, K], bf16)
            nc.sync.dma_start(cTs[:], cTs_d[:])
            pre = const.tile([1, 2, K], bf16)
            nc.sync.dma_start(pre[:], pre_d[:])
            ones1 = const.tile([1, 128], bf16)
            nc.gpsimd.memset(ones1[:], 1.0)
            ident = const.tile([128, 128], f32)
            make_identity(nc, ident[:])

            driftacc = accp.tile([128, n_tiles * CH], f32)

            # G matmul N-halves (<=512 free dim per PSUM bank)
            halves = [(0, 512), (512, K)]

            for i in range(n_tiles):
                # ---- load inputs (feature-major bf16) ----------------------
                xts = []
                xnts = []
                for kc in range(KC1):
                    t = xin.tile([128, TOK_TILE], bf16, tag="xin")
                    nc.sync.dma_start(t[:], xT[i, kc])
                    xts.append(t)
                for kc in range(KC1):
                    t = xin.tile([128, TOK_TILE], bf16, tag="xin")
                    nc.sync.dma_start(t[:], xnT[i, kc])
                    xnts.append(t)

                # ---- layer 1 (both passes) + relu --------------------------
                h1b, h2b = [], []
                for src, dst in ((xts, h1b), (xnts, h2b)):
                    for fc in range(FC1):
                        hT = mmp.tile([128, TOK_TILE], mybir.dt.float32,
                                      tag="mm")
                        for kc in range(KC1):
                            nc.tensor.matmul(
                                hT[:],
                                lhsT=W1s[:, kc, fc * 128:(fc + 1) * 128],
                                rhs=src[kc][:],
                                start=(kc == 0),
                                stop=(kc == KC1 - 1),
                            )
                        hb = hsb.tile([128, TOK_TILE], bf16, tag="h")
                        nc.scalar.activation(
                            hb[:], hT[:], mybir.ActivationFunctionType.Relu,
                            bias=b1s[:, fc:fc + 1],
                        )
                        dst.append(hb)

                # ---- layer 2, clean pass (feature-major, bias folded) ------
                zT = mmp.tile([128, TOK_TILE], mybir.dt.float32, tag="mm")
                for kc in range(FC1):
                    nc.tensor.matmul(
                        zT[:], lhsT=W2s[:, kc, :], rhs=h1b[kc][:],
                        start=(kc == 0), stop=(kc == FC1 - 1),
                    )
                z1b = zsb.tile([128, TOK_TILE], bf16, tag="z1")
                nc.scalar.activation(z1b[:], zT[:],
                                     mybir.ActivationFunctionType.Copy)

                cls4 = small.tile([128, CH, 8], u32, tag="cls")
                d2c = small.tile([128, CH], mybir.dt.float32, tag="d2")

                for c in range(CH):
                    csl = slice(c * 128, (c + 1) * 128)

                    # ---- scores G = z1.c_j + pre_j  (PSUM, fp32) -----------
                    G = gpp.tile([128, 1024], mybir.dt.float32, tag="G")
                    for lo, hi in halves:
                        nc.tensor.matmul(
                            G[:, lo:lo + (hi - lo)],
                            lhsT=ones1[:], rhs=pre[:, 0, lo:hi],
                            start=True, stop=False,
                        )
                        nc.tensor.matmul(
                            G[:, lo:lo + (hi - lo)],
                            lhsT=z1b[:, csl], rhs=cTs[:, lo:hi],
                            start=False, stop=True,
                        )

                    # ---- argmax over centroids -----------------------------
                    m8 = small.tile([128, 8], mybir.dt.float32, tag="m8")
                    nc.vector.max(out=m8[:], in_=G[:, :K])
                    nc.vector.max_index(
                        out=cls4[:, c, :], in_max=m8[:], in_values=G[:, :K],
                    )
                    if debug_taps:
                        nc.sync.dma_start(m8_dbg[i, :, c, :], m8[:])

                # ---- gather [c_j - b2, A, B, pad] rows by cls --------------
                # One indirect DMA per 128-token chunk with [128, 1] offsets:
                # multi-offset-per-partition gathers pair offsets to output
                # rows in a different order on HW than in the simulator.
                tabsel = tabp.tile([128, CH, TAB_W], mybir.dt.float32,
                                   tag="tab")
                for c in range(CH):
                    nc.gpsimd.indirect_dma_start(
                        out=tabsel[:, c, :],
                        out_offset=None,
                        in_=ctab[:],
                        in_offset=bass.IndirectOffsetOnAxis(
                            ap=cls4[:, c, 0:1], axis=0),
                        bounds_check=K - 1,
                        oob_is_err=False,
                    )

                for c in range(CH):
                    csl = slice(c * 128, (c + 1) * 128)
                    z2r = z2rp.tile([128, D_LAT], mybir.dt.float32, tag="z2r")
                    for kc in range(FC1):
                        nc.tensor.matmul(
                            z2r[:], lhsT=h2b[kc][:, csl], rhs=W2s[:, kc, :],
                            start=(kc == 0), stop=(kc == FC1 - 1),
                        )
                    # diff = z2r - csel   (bf16)
                    diff = small.tile([128, D_LAT], bf16, tag="diff")
                    nc.vector.scalar_tensor_tensor(
                        out=diff[:],
                        in0=tabsel[:, c, 0:D_LAT],
                        scalar=-1.0,
                        in1=z2r[:],
                        op0=mybir.AluOpType.mult,
                        op1=mybir.AluOpType.add,
                    )
                    # d2 = sum(diff*diff)   (ScalarE: Square with accumulate)
                    junk = small.tile([128, D_LAT], bf16, tag="junk")
                    nc.scalar.activation(
                        junk[:], diff[:],
                        mybir.ActivationFunctionType.Square,
                        accum_out=d2c[:, c:c + 1],
                    )

                # ---- drift = (d2 > A) | (d2 < B) ---------------------------
                ga = small.tile([128, CH], mybir.dt.float32, tag="ga")
                gb = small.tile([128, CH], mybir.dt.float32, tag="gb")
                nc.vector.tensor_tensor(
                    out=ga[:], in0=d2c[:], in1=tabsel[:, :, 128],
                    op=mybir.AluOpType.is_gt,
                )
                nc.vector.tensor_tensor(
                    out=gb[:], in0=d2c[:], in1=tabsel[:, :, 129],
                    op=mybir.AluOpType.is_lt,
                )
                nc.vector.tensor_tensor(
                    out=driftacc[:, i * CH:(i + 1) * CH],
                    in0=ga[:], in1=gb[:], op=mybir.AluOpType.max,
                )
                if debug_taps:
                    nc.sync.dma_start(cls_dbg[i], cls4[:])
                    nc.sync.dma_start(d2_dbg[i], d2c[:])
                    nc.sync.dma_start(tab_dbg[i], tabsel[:])

            # ---- transpose [128, n_tiles*CH] -> token order and store ------
            ncols = n_tiles * CH
            tpsum = z2rp.tile([128, 128], mybir.dt.float32, tag="z2r")
            nc.tensor.transpose(tpsum[:ncols, :], driftacc[:, :ncols],
                                ident[:])
            drift_i = small.tile([128, 128], i32, tag="drifti")
            nc.vector.tensor_copy(out=drift_i[:ncols, :], in_=tpsum[:ncols, :])
            nc.sync.dma_start(
                drift_d.rearrange("(a b) -> a b", b=128),
                drift_i[:ncols, :],
            )

    nc.compile()
    return nc


def prep_inputs(x, noise, W1, b1, W2, b2, centroid, dis_median, mad,
                n_tiles=BS // TOK_TILE, n_cores=N_CORES):
    """Host-side preparation of per-core input maps."""
    bs = n_tiles * TOK_TILE
    x = np.asarray(x, dtype=np.float32)
    noise = np.asarray(noise, dtype=np.float32)
    W1 = np.asarray(W1, dtype=np.float32)
    b1 = np.asarray(b1, dtype=np.float32)
    W2 = np.asarray(W2, dtype=np.float32)
    b2 = np.asarray(b2, dtype=np.float32)
    centroid = np.asarray(centroid, dtype=np.float32)
    dis_median = np.asarray(dis_median, dtype=np.float32)
    mad = np.asarray(mad, dtype=np.float32)

    xn = x + noise

    W1s = np.ascontiguousarray(
        W1.reshape(KC1, 128, H).transpose(1, 0, 2)).astype(BF16)
    W2s = np.ascontiguousarray(
        W2.reshape(FC1, 128, D_LAT).transpose(1, 0, 2)).astype(BF16)
    b1s = np.ascontiguousarray(b1.reshape(FC1, 128).T)
    cTs = np.ascontiguousarray(centroid.T).astype(BF16)

    c2 = (centroid * centroid).sum(1)
    pre_f = PRE_SHIFT - 0.5 * c2 + centroid @ b2
    pre_hi = pre_f.astype(BF16)
    pre_lo = (pre_f - pre_hi.astype(np.float32)).astype(BF16)
    pre = np.ascontiguousarray(
        np.stack([pre_hi, pre_lo])[None, :, :])            # [1, 2, K]

    hi = dis_median + MAD_THRESHOLD * mad
    lo = dis_median - MAD_THRESHOLD * mad
    A = (hi * hi).astype(np.float32)
    Bv = np.where(lo > 0, lo * lo, -1.0).astype(np.float32)
    ctab = np.zeros((K, TAB_W), dtype=np.float32)
    ctab[:, :D_LAT] = centroid - b2[None, :]
    ctab[:, 128] = A
    ctab[:, 129] = Bv

    def shard_T(a, core):
        s = a[core * bs:(core + 1) * bs].astype(BF16)       # [bs, 512]
        sT = s.T                                            # [512, bs]
        blk = sT.reshape(KC1, 128, n_tiles, TOK_TILE).transpose(2, 0, 1, 3)
        return np.ascontiguousarray(blk)

    in_maps = []
    for core in range(n_cores):
        in_maps.append({
            "xT": shard_T(x, core),
            "xnT": shard_T(xn, core),
            "W1s": W1s,
            "W2s": W2s,
            "b1s": b1s,
            "cTs": cTs,
            "pre": pre,
            "ctab": ctab,
        })
    return in_maps


_BUILD_CACHE = {}


def kernel(x, noise, W1, b1, W2, b2, centroid, dis_median, mad):
    from concourse.bass_utils import run_bass_kernel_spmd

    nc = _BUILD_CACHE.get("nc")
    if nc is None:
        nc = _BUILD_CACHE["nc"] = build_program()
    in_maps = prep_inputs(x, noise, W1, b1, W2, b2, centroid,
                          dis_median, mad)
    res = run_bass_kernel_spmd(nc, in_maps, core_ids=list(range(N_CORES)))
    out = np.concatenate([r["drift"] for r in res.results])
    return out.astype(np.int32)


# revision 23
# speedup vs baseline: 309.2457x; 1.0079x over previous
"""Trainium2 Bass kernel for nn_Detector (retrieval_knn drift detector).

Pipeline (per token):
    z1 = relu(x @ W1 + b1) @ W2 + b2
    cls = argmin_j ||z1 - centroid_j||
    z2 = relu((x+noise) @ W1 + b1) @ W2 + b2
    dis = ||z2 - centroid_cls||
    drift = |dis - med_cls| / mad_cls > 3.5

Strategy: pure data-parallel over 8 NeuronCores (8192 tokens each).
On-chip activations are feature-major ([feat, tok]) so the contraction dim
sits on partitions.  All matmuls in bf16 (output drift bits have >4.5 sigma
margin vs the 3.5 threshold, verified against the fp32 reference).

Algebraic rewrites baked in on the host:
  - argmin_j ||z1-c_j||^2 == argmax_j (z1.c_j - 0.5||c_j||^2).  The per-j
    constant (-0.5||c_j||^2 + b2.c_j) is added via a rank-1 matmul preload
    into PSUM (ones (x) pre_j, bf16), which also folds away the b2 bias of
    the first encoder pass.
  - drift = (d2 > A_cls) | (d2 < B_cls) with d2 = ||z2' - (c_cls - b2)||^2,
    z2' the bias-free second encoding, A = (med+3.5*mad)^2 and
    B = (med-3.5*mad)^2 if med > 3.5*mad else -1.  No sqrt, no division,
    no med/mad gathers.
  - gather table rows [c_j - b2 (128 f32), A_j, B_j, pad, pad] fetched by
    one indirect DMA per 512-token tile.
"""

import numpy as np
import ml_dtypes

import concourse.bass as bass
import concourse.bacc as bacc
import concourse.mybir as mybir
import concourse.tile as tile
from concourse.masks import make_identity

BF16 = ml_dtypes.bfloat16

B, D_IN, H, D_LAT, K = 65536, 512, 256, 128, 1000
MAD_THRESHOLD = 3.5
N_CORES = 8
BS = B // N_CORES            # tokens per core
TOK_TILE = 512               # tokens per pipeline tile
KC1 = D_IN // 128            # 4  K-chunks for layer 1
FC1 = H // 128               # 2  feature chunks of the hidden layer
TAB_W = 132                  # gather-table row width (128 + A + B + 2 pad)
PRE_SHIFT = 0.0              # no offset: |pre| stays small so the single
                             # bf16 rank-1 preload keeps ~0.1 precision


def build_program(n_tiles=BS // TOK_TILE, enable_asserts=False,
                  debug_taps=False):
    """Build the per-core Bass program.  Returns (nc, names) where names maps
    logical tensors to dram tensor names."""
    bs = n_tiles * TOK_TILE
    nc = bacc.Bacc(
        "TRN2",
        target_bir_lowering=False,
        debug=False,
        enable_asserts=enable_asserts,
        num_devices=N_CORES,
    )
    f32, bf16, i32, u32 = (
        mybir.dt.float32, mybir.dt.bfloat16, mybir.dt.int32, mybir.dt.uint32,
    )

    xT = nc.dram_tensor("xT", [n_tiles, KC1, 128, TOK_TILE], bf16,
                        kind="ExternalInput").ap()
    xnT = nc.dram_tensor("xnT", [n_tiles, KC1, 128, TOK_TILE], bf16,
                         kind="ExternalInput").ap()
    W1s_d = nc.dram_tensor("W1s", [128, KC1, H], bf16, kind="ExternalInput").ap()
    W2s_d = nc.dram_tensor("W2s", [128, FC1, D_LAT], bf16,
                           kind="ExternalInput").ap()
    b1s_d = nc.dram_tensor("b1s", [128, FC1], f32, kind="ExternalInput").ap()
    cTs_d = nc.dram_tensor("cTs", [128, K], bf16, kind="ExternalInput").ap()
    pre_d = nc.dram_tensor("pre", [1, 2, K], bf16, kind="ExternalInput").ap()
    ctab = nc.dram_tensor("ctab", [K, TAB_W], f32, kind="ExternalInput").ap()
    drift_d = nc.dram_tensor("drift", [bs], i32, kind="ExternalOutput").ap()

    CH = TOK_TILE // 128     # 4 token chunks per tile
    if debug_taps:
        cls_dbg = nc.dram_tensor("cls_dbg", [n_tiles, 128, CH, 8], u32,
                                 kind="ExternalOutput").ap()
        d2_dbg = nc.dram_tensor("d2_dbg", [n_tiles, 128, CH], f32,
                                kind="ExternalOutput").ap()
        m8_dbg = nc.dram_tensor("m8_dbg", [n_tiles, 128, CH, 8], f32,
                                kind="ExternalOutput").ap()
        tab_dbg = nc.dram_tensor("tab_dbg", [n_tiles, 128, CH, TAB_W], f32,
                                 kind="ExternalOutput").ap()

    with tile.TileContext(nc) as tc:
        with (
            tc.tile_pool(name="const", bufs=1) as const,
            tc.tile_pool(name="xin", bufs=16) as xin,
            tc.tile_pool(name="hsb", bufs=8) as hsb,
            tc.tile_pool(name="zsb", bufs=2) as zsb,
            tc.tile_pool(name="small", bufs=4) as small,
            tc.tile_pool(name="tab", bufs=2) as tabp,
            tc.tile_pool(name="acc", bufs=1) as accp,
            tc.tile_pool(name="mm", bufs=3, space="PSUM") as mmp,
            tc.tile_pool(name="gp", bufs=2, space="PSUM") as gpp,
            tc.tile_pool(name="z2r", bufs=1, space="PSUM") as z2rp,
        ):
            # ---- constants -------------------------------------------------
            W1s = const.tile([128, KC1, H], bf16)
            nc.sync.dma_start(W1s[:], W1s_d[:])
            W2s = const.tile([128, FC1, D_LAT], bf16)
            nc.sync.dma_start(W2s[:], W2s_d[:])
            b1s = const.tile([128, FC1], f32)
            nc.sync.dma_start(b1s[:], b1s_d[:])
            cTs = const.tile([128, K], bf16)
            nc.sync.dma_start(cTs[:], cTs_d[:])
            pre = const.tile([1, 2, K], bf16)
            nc.sync.dma_start(pre[:], pre_d[:])
            ones1 = const.tile([1, 128], bf16)
            nc.gpsimd.memset(ones1[:], 1.0)
            ident = const.tile([128, 128], f32)
            make_identity(nc, ident[:])

            driftacc = accp.tile([128, n_tiles * CH], f32)

            # G matmul N-halves (<=512 free dim per PSUM bank)
            halves = [(0, 512), (512, K)]

            for i in range(n_tiles):
                # ---- load inputs (feature-major bf16) ----------------------
                xts = []
                xnts = []
                for kc in range(KC1):
                    t = xin.tile([128, TOK_TILE], bf16, tag="xin")
                    nc.sync.dma_start(t[:], xT[i, kc])
                    xts.append(t)
                for kc in range(KC1):
                    t = xin.tile([128, TOK_TILE], bf16, tag="xin")
                    nc.sync.dma_start(t[:], xnT[i, kc])
                    xnts.append(t)

                # ---- layer 1 (both passes) + relu --------------------------
                h1b, h2b = [], []
                for src, dst in ((xts, h1b), (xnts, h2b)):
                    for fc in range(FC1):
                        hT = mmp.tile([128, TOK_TILE], mybir.dt.float32,
                                      tag="mm")
                        for kc in range(KC1):
                            nc.tensor.matmul(
                                hT[:],
                                lhsT=W1s[:, kc, fc * 128:(fc + 1) * 128],
                                rhs=src[kc][:],
                                start=(kc == 0),
                                stop=(kc == KC1 - 1),
                            )
                        hb = hsb.tile([128, TOK_TILE], bf16, tag="h")
                        nc.scalar.activation(
                            hb[:], hT[:], mybir.ActivationFunctionType.Relu,
                            bias=b1s[:, fc:fc + 1],
                        )
                        dst.append(hb)

                # ---- layer 2, clean pass (feature-major, bias folded) ------
                zT = mmp.tile([128, TOK_TILE], mybir.dt.float32, tag="mm")
                for kc in range(FC1):
                    nc.tensor.matmul(
                        zT[:], lhsT=W2s[:, kc, :], rhs=h1b[kc][:],
                        start=(kc == 0), stop=(kc == FC1 - 1),
                    )
                z1b = zsb.tile([128, TOK_TILE], bf16, tag="z1")
                nc.scalar.activation(z1b[:], zT[:],
                                     mybir.ActivationFunctionType.Copy)

                cls4 = small.tile([128, CH, 8], u32, tag="cls")
                d2c = small.tile([128, CH], mybir.dt.float32, tag="d2")

                for c in range(CH):
                    csl = slice(c * 128, (c + 1) * 128)

                    # ---- scores G = z1.c_j + pre_j  (PSUM, fp32) -----------
                    G = gpp.tile([128, 1024], mybir.dt.float32, tag="G")
                    for lo, hi in halves:
                        nc.tensor.matmul(
                            G[:, lo:lo + (hi - lo)],
                            lhsT=ones1[:], rhs=pre[:, 0, lo:hi],
                            start=True, stop=False,
                        )
                        nc.tensor.matmul(
                            G[:, lo:lo + (hi - lo)],
                            lhsT=z1b[:, csl], rhs=cTs[:, lo:hi],
                            start=False, stop=True,
                        )

                    # ---- argmax over centroids -----------------------------
                    m8 = small.tile([128, 8], mybir.dt.float32, tag="m8")
                    nc.vector.max(out=m8[:], in_=G[:, :K])
                    nc.vector.max_index(
                        out=cls4[:, c, :], in_max=m8[:], in_values=G[:, :K],
                    )
                    if debug_taps:
                        nc.sync.dma_start(m8_dbg[i, :, c, :], m8[:])

                # ---- gather [c_j - b2, A, B, pad] rows by cls --------------
                # One indirect DMA per 128-token chunk with [128, 1] offsets:
                # multi-offset-per-partition gathers pair offsets to output
                # rows in a different order on HW than in the simulator.
                tabsel = tabp.tile([128, CH, TAB_W], mybir.dt.float32,
                                   tag="tab")
                for c in range(CH):
                    nc.gpsimd.indirect_dma_start(
                        out=tabsel[:, c, :],
                        out_offset=None,
                        in_=ctab[:],
                        in_offset=bass.IndirectOffsetOnAxis(
                            ap=cls4[:, c, 0:1], axis=0),
                        bounds_check=K - 1,
                        oob_is_err=False,
                    )

                # all 4 z2-row chunks into one PSUM bank so the diff is a
                # single DVE op over [128, 512]
                z2rb = z2rp.tile([128, CH, D_LAT], mybir.dt.float32,
                                 tag="z2r")
                for c in range(CH):
                    csl = slice(c * 128, (c + 1) * 128)
                    for kc in range(FC1):
                        nc.tensor.matmul(
                            z2rb[:, c, :], lhsT=h2b[kc][:, csl],
                            rhs=W2s[:, kc, :],
                            start=(kc == 0), stop=(kc == FC1 - 1),
                        )
                diffb = small.tile([128, CH, D_LAT], bf16, tag="diff")
                nc.vector.scalar_tensor_tensor(
                    out=diffb[:],
                    in0=tabsel[:, :, 0:D_LAT],
                    scalar=-1.0,
                    in1=z2rb[:],
                    op0=mybir.AluOpType.mult,
                    op1=mybir.AluOpType.add,
                )
                # d2 = sum(diff*diff)   (ScalarE: Square with accumulate)
                for c in range(CH):
                    junk = small.tile([128, D_LAT], bf16, tag="junk")
                    nc.scalar.activation(
                        junk[:], diffb[:, c, :],
                        mybir.ActivationFunctionType.Square,
                        accum_out=d2c[:, c:c + 1],
                    )

                # ---- drift = (d2 > A) | (d2 < B) ---------------------------
                ga = small.tile([128, CH], mybir.dt.float32, tag="ga")
                gb = small.tile([128, CH], mybir.dt.float32, tag="gb")
                nc.vector.tensor_tensor(
                    out=ga[:], in0=d2c[:], in1=tabsel[:, :, 128],
                    op=mybir.AluOpType.is_gt,
                )
                nc.vector.tensor_tensor(
                    out=gb[:], in0=d2c[:], in1=tabsel[:, :, 129],
                    op=mybir.AluOpType.is_lt,
                )
                nc.vector.tensor_tensor(
                    out=driftacc[:, i * CH:(i + 1) * CH],
                    in0=ga[:], in1=gb[:], op=mybir.AluOpType.max,
                )
                if debug_taps:
                    nc.sync.dma_start(cls_dbg[i], cls4[:])
                    nc.sync.dma_start(d2_dbg[i], d2c[:])
                    nc.sync.dma_start(tab_dbg[i], tabsel[:])

            # ---- transpose [128, n_tiles*CH] -> token order and store ------
            ncols = n_tiles * CH
            tpsum = z2rp.tile([128, 128], mybir.dt.float32, tag="z2r")
            nc.tensor.transpose(tpsum[:ncols, :], driftacc[:, :ncols],
                                ident[:])
            drift_i = small.tile([128, 128], i32, tag="drifti")
            nc.vector.tensor_copy(out=drift_i[:ncols, :], in_=tpsum[:ncols, :])
            nc.sync.dma_start(
                drift_d.rearrange("(a b) -> a b", b=128),
                drift_i[:ncols, :],
            )

    nc.compile()
    return nc


def prep_inputs(x, noise, W1, b1, W2, b2, centroid, dis_median, mad,
                n_tiles=BS // TOK_TILE, n_cores=N_CORES):
    """Host-side preparation of per-core input maps."""
    bs = n_tiles * TOK_TILE
    x = np.asarray(x, dtype=np.float32)
    noise = np.asarray(noise, dtype=np.float32)
    W1 = np.asarray(W1, dtype=np.float32)
    b1 = np.asarray(b1, dtype=np.float32)
    W2 = np.asarray(W2, dtype=np.float32)
    b2 = np.asarray(b2, dtype=np.float32)
    centroid = np.asarray(centroid, dtype=np.float32)
    dis_median = np.asarray(dis_median, dtype=np.float32)
    mad = np.asarray(mad, dtype=np.float32)

    xn = x + noise

    W1s = np.ascontiguousarray(
        W1.reshape(KC1, 128, H).transpose(1, 0, 2)).astype(BF16)
    W2s = np.ascontiguousarray(
        W2.reshape(FC1, 128, D_LAT).transpose(1, 0, 2)).astype(BF16)
    b1s = np.ascontiguousarray(b1.reshape(FC1, 128).T)
    cTs = np.ascontiguousarray(centroid.T).astype(BF16)

    c2 = (centroid * centroid).sum(1)
    pre_f = PRE_SHIFT - 0.5 * c2 + centroid @ b2
    pre_hi = pre_f.astype(BF16)
    pre_lo = (pre_f - pre_hi.astype(np.float32)).astype(BF16)
    pre = np.ascontiguousarray(
        np.stack([pre_hi, pre_lo])[None, :, :])            # [1, 2, K]

    hi = dis_median + MAD_THRESHOLD * mad
    lo = dis_median - MAD_THRESHOLD * mad
    A = (hi * hi).astype(np.float32)
    Bv = np.where(lo > 0, lo * lo, -1.0).astype(np.float32)
    ctab = np.zeros((K, TAB_W), dtype=np.float32)
    ctab[:, :D_LAT] = centroid - b2[None, :]
    ctab[:, 128] = A
    ctab[:, 129] = Bv

    def shard_T(a, core):
        s = a[core * bs:(core + 1) * bs].astype(BF16)       # [bs, 512]
        sT = s.T                                            # [512, bs]
        blk = sT.reshape(KC1, 128, n_tiles, TOK_TILE).transpose(2, 0, 1, 3)
        return np.ascontiguousarray(blk)

    in_maps = []
    for core in range(n_cores):
        in_maps.append({
            "xT": shard_T(x, core),
            "xnT": shard_T(xn, core),
            "W1s": W1s,
            "W2s": W2s,
            "b1s": b1s,
            "cTs": cTs,
            "pre": pre,
            "ctab": ctab,
        })
    return in_maps


_BUILD_CACHE = {}


def kernel(x, noise, W1, b1, W2, b2, centroid, dis_median, mad):
    from concourse.bass_utils import run_bass_kernel_spmd

    nc = _BUILD_CACHE.get("nc")
    if nc is None:
        nc = _BUILD_CACHE["nc"] = build_program()
    in_maps = prep_inputs(x, noise, W1, b1, W2, b2, centroid,
                          dis_median, mad)
    res = run_bass_kernel_spmd(nc, in_maps, core_ids=list(range(N_CORES)))
    out = np.concatenate([r["drift"] for r in res.results])
    return out.astype(np.int32)


# revision 25
# speedup vs baseline: 312.5892x; 1.0108x over previous
"""Trainium2 Bass kernel for nn_Detector (retrieval_knn drift detector).

Pipeline (per token):
    z1 = relu(x @ W1 + b1) @ W2 + b2
    cls = argmin_j ||z1 - centroid_j||
    z2 = relu((x+noise) @ W1 + b1) @ W2 + b2
    dis = ||z2 - centroid_cls||
    drift = |dis - med_cls| / mad_cls > 3.5

Strategy: pure data-parallel over 8 NeuronCores (8192 tokens each).
On-chip activations are feature-major ([feat, tok]) so the contraction dim
sits on partitions.  All matmuls in bf16 (output drift bits have >4.5 sigma
margin vs the 3.5 threshold, verified against the fp32 reference).

Algebraic rewrites baked in on the host:
  - argmin_j ||z1-c_j||^2 == argmax_j (z1.c_j - 0.5||c_j||^2).  The per-j
    constant (-0.5||c_j||^2 + b2.c_j) is added via a rank-1 matmul preload
    into PSUM (ones (x) pre_j, bf16), which also folds away the b2 bias of
    the first encoder pass.
  - drift = (d2 > A_cls) | (d2 < B_cls) with d2 = ||z2' - (c_cls - b2)||^2,
    z2' the bias-free second encoding, A = (med+3.5*mad)^2 and
    B = (med-3.5*mad)^2 if med > 3.5*mad else -1.  No sqrt, no division,
    no med/mad gathers.
  - gather table rows [c_j - b2 (128 f32), A_j, B_j, pad, pad] fetched by
    one indirect DMA per 512-token tile.
"""

import numpy as np
import ml_dtypes

import concourse.bass as bass
import concourse.bacc as bacc
import concourse.mybir as mybir
import concourse.tile as tile
from concourse.masks import make_identity

BF16 = ml_dtypes.bfloat16

B, D_IN, H, D_LAT, K = 65536, 512, 256, 128, 1000
MAD_THRESHOLD = 3.5
N_CORES = 8
BS = B // N_CORES            # tokens per core
TOK_TILE = 512               # tokens per pipeline tile
KC1 = D_IN // 128            # 4  K-chunks for layer 1
FC1 = H // 128               # 2  feature chunks of the hidden layer
TAB_W = 132                  # gather-table row width (128 + A + B + 2 pad)
PRE_SHIFT = 0.0              # no offset: |pre| stays small so the single
                             # bf16 rank-1 preload keeps ~0.1 precision


def build_program(n_tiles=BS // TOK_TILE, enable_asserts=False,
                  debug_taps=False):
    """Build the per-core Bass program.  Returns (nc, names) where names maps
    logical tensors to dram tensor names."""
    bs = n_tiles * TOK_TILE
    nc = bacc.Bacc(
        "TRN2",
        target_bir_lowering=False,
        debug=False,
        enable_asserts=enable_asserts,
        num_devices=N_CORES,
    )
    f32, bf16, i32, u32 = (
        mybir.dt.float32, mybir.dt.bfloat16, mybir.dt.int32, mybir.dt.uint32,
    )

    xT = nc.dram_tensor("xT", [n_tiles, KC1, 128, TOK_TILE], bf16,
                        kind="ExternalInput").ap()
    xnT = nc.dram_tensor("xnT", [n_tiles, KC1, 128, TOK_TILE], bf16,
                         kind="ExternalInput").ap()
    W1s_d = nc.dram_tensor("W1s", [128, KC1, H], bf16, kind="ExternalInput").ap()
    W2s_d = nc.dram_tensor("W2s", [128, FC1, D_LAT], bf16,
                           kind="ExternalInput").ap()
    b1s_d = nc.dram_tensor("b1s", [128, FC1], f32, kind="ExternalInput").ap()
    cTs_d = nc.dram_tensor("cTs", [128, K], bf16, kind="ExternalInput").ap()
    pre_d = nc.dram_tensor("pre", [1, 2, K], bf16, kind="ExternalInput").ap()
    ctab = nc.dram_tensor("ctab", [K, TAB_W], f32, kind="ExternalInput").ap()
    drift_d = nc.dram_tensor("drift", [bs], i32, kind="ExternalOutput").ap()

    CH = TOK_TILE // 128     # 4 token chunks per tile
    if debug_taps:
        cls_dbg = nc.dram_tensor("cls_dbg", [n_tiles, 128, CH, 8], u32,
                                 kind="ExternalOutput").ap()
        d2_dbg = nc.dram_tensor("d2_dbg", [n_tiles, 128, CH], f32,
                                kind="ExternalOutput").ap()
        m8_dbg = nc.dram_tensor("m8_dbg", [n_tiles, 128, CH, 8], f32,
                                kind="ExternalOutput").ap()
        tab_dbg = nc.dram_tensor("tab_dbg", [n_tiles, 128, CH, TAB_W], f32,
                                 kind="ExternalOutput").ap()

    with tile.TileContext(nc) as tc:
        with (
            tc.tile_pool(name="const", bufs=1) as const,
            tc.tile_pool(name="xin", bufs=16) as xin,
            tc.tile_pool(name="hsb", bufs=8) as hsb,
            tc.tile_pool(name="zsb", bufs=2) as zsb,
            tc.tile_pool(name="small", bufs=4) as small,
            tc.tile_pool(name="tab", bufs=2) as tabp,
            tc.tile_pool(name="acc", bufs=1) as accp,
            tc.tile_pool(name="mm", bufs=3, space="PSUM") as mmp,
            tc.tile_pool(name="gp", bufs=2, space="PSUM") as gpp,
            tc.tile_pool(name="z2r", bufs=1, space="PSUM") as z2rp,
        ):
            # ---- constants -------------------------------------------------
            W1s = const.tile([128, KC1, H], bf16)
            nc.sync.dma_start(W1s[:], W1s_d[:])
            W2s = const.tile([128, FC1, D_LAT], bf16)
            nc.sync.dma_start(W2s[:], W2s_d[:])
            b1s = const.tile([128, FC1], f32)
            nc.sync.dma_start(b1s[:], b1s_d[:])
            cTs = const.tile([128, K], bf16)
            nc.sync.dma_start(cTs[:], cTs_d[:])
            pre = const.tile([1, 2, K], bf16)
            nc.sync.dma_start(pre[:], pre_d[:])
            ones1 = const.tile([1, 128], bf16)
            nc.gpsimd.memset(ones1[:], 1.0)
            ident = const.tile([128, 128], f32)
            make_identity(nc, ident[:])

            driftacc = accp.tile([128, n_tiles * CH], f32)

            # G matmul N-halves (<=512 free dim per PSUM bank)
            halves = [(0, 512), (512, K)]

            for i in range(n_tiles):
                # ---- load inputs (feature-major bf16) ----------------------
                xts = []
                xnts = []
                for kc in range(KC1):
                    t = xin.tile([128, TOK_TILE], bf16, tag="xin")
                    nc.sync.dma_start(t[:], xT[i, kc])
                    xts.append(t)
                for kc in range(KC1):
                    t = xin.tile([128, TOK_TILE], bf16, tag="xin")
                    nc.sync.dma_start(t[:], xnT[i, kc])
                    xnts.append(t)

                # ---- layer 1, clean pass only (noise pass issued later,
                # after the argmax chain, for earlier DVE starts) ------------
                h1b, h2b = [], []
                for fc in range(FC1):
                    hT = mmp.tile([128, TOK_TILE], mybir.dt.float32,
                                  tag="mm")
                    for kc in range(KC1):
                        nc.tensor.matmul(
                            hT[:],
                            lhsT=W1s[:, kc, fc * 128:(fc + 1) * 128],
                            rhs=xts[kc][:],
                            start=(kc == 0),
                            stop=(kc == KC1 - 1),
                        )
                    hb = hsb.tile([128, TOK_TILE], bf16, tag="h")
                    nc.scalar.activation(
                        hb[:], hT[:], mybir.ActivationFunctionType.Relu,
                        bias=b1s[:, fc:fc + 1],
                    )
                    h1b.append(hb)

                # ---- layer 2, clean pass (feature-major, bias folded) ------
                zT = mmp.tile([128, TOK_TILE], mybir.dt.float32, tag="mm")
                for kc in range(FC1):
                    nc.tensor.matmul(
                        zT[:], lhsT=W2s[:, kc, :], rhs=h1b[kc][:],
                        start=(kc == 0), stop=(kc == FC1 - 1),
                    )
                z1b = zsb.tile([128, TOK_TILE], bf16, tag="z1")
                nc.scalar.activation(z1b[:], zT[:],
                                     mybir.ActivationFunctionType.Copy)

                cls4 = small.tile([128, CH, 8], u32, tag="cls")
                d2c = small.tile([128, CH], mybir.dt.float32, tag="d2")

                for c in range(CH):
                    csl = slice(c * 128, (c + 1) * 128)

                    # ---- scores G = z1.c_j + pre_j  (PSUM, fp32) -----------
                    G = gpp.tile([128, 1024], mybir.dt.float32, tag="G")
                    for lo, hi in halves:
                        nc.tensor.matmul(
                            G[:, lo:lo + (hi - lo)],
                            lhsT=ones1[:], rhs=pre[:, 0, lo:hi],
                            start=True, stop=False,
                        )
                        nc.tensor.matmul(
                            G[:, lo:lo + (hi - lo)],
                            lhsT=z1b[:, csl], rhs=cTs[:, lo:hi],
                            start=False, stop=True,
                        )

                    # ---- argmax over centroids -----------------------------
                    m8 = small.tile([128, 8], mybir.dt.float32, tag="m8")
                    nc.vector.max(out=m8[:], in_=G[:, :K])
                    nc.vector.max_index(
                        out=cls4[:, c, :], in_max=m8[:], in_values=G[:, :K],
                    )
                    if debug_taps:
                        nc.sync.dma_start(m8_dbg[i, :, c, :], m8[:])

                # ---- gather [c_j - b2, A, B, pad] rows by cls --------------
                # One indirect DMA per 128-token chunk with [128, 1] offsets:
                # multi-offset-per-partition gathers pair offsets to output
                # rows in a different order on HW than in the simulator.
                # ---- layer 1, noise pass (not needed until the z2 rows) ----
                for fc in range(FC1):
                    hT = mmp.tile([128, TOK_TILE], mybir.dt.float32,
                                  tag="mm")
                    for kc in range(KC1):
                        nc.tensor.matmul(
                            hT[:],
                            lhsT=W1s[:, kc, fc * 128:(fc + 1) * 128],
                            rhs=xnts[kc][:],
                            start=(kc == 0),
                            stop=(kc == KC1 - 1),
                        )
                    hb = hsb.tile([128, TOK_TILE], bf16, tag="h")
                    nc.scalar.activation(
                        hb[:], hT[:], mybir.ActivationFunctionType.Relu,
                        bias=b1s[:, fc:fc + 1],
                    )
                    h2b.append(hb)

                tabsel = tabp.tile([128, CH, TAB_W], mybir.dt.float32,
                                   tag="tab")
                for c in range(CH):
                    nc.gpsimd.indirect_dma_start(
                        out=tabsel[:, c, :],
                        out_offset=None,
                        in_=ctab[:],
                        in_offset=bass.IndirectOffsetOnAxis(
                            ap=cls4[:, c, 0:1], axis=0),
                        bounds_check=K - 1,
                        oob_is_err=False,
                    )

                # all 4 z2-row chunks into one PSUM bank so the diff is a
                # single DVE op over [128, 512]
                z2rb = z2rp.tile([128, CH, D_LAT], mybir.dt.float32,
                                 tag="z2r")
                for c in range(CH):
                    csl = slice(c * 128, (c + 1) * 128)
                    for kc in range(FC1):
                        nc.tensor.matmul(
                            z2rb[:, c, :], lhsT=h2b[kc][:, csl],
                            rhs=W2s[:, kc, :],
                            start=(kc == 0), stop=(kc == FC1 - 1),
                        )
                diffb = small.tile([128, CH, D_LAT], bf16, tag="diff")
                nc.vector.scalar_tensor_tensor(
                    out=diffb[:],
                    in0=tabsel[:, :, 0:D_LAT],
                    scalar=-1.0,
                    in1=z2rb[:],
                    op0=mybir.AluOpType.mult,
                    op1=mybir.AluOpType.add,
                )
                # d2 = sum(diff*diff)   (ScalarE: Square with accumulate)
                for c in range(CH):
                    junk = small.tile([128, D_LAT], bf16, tag="junk")
                    nc.scalar.activation(
                        junk[:], diffb[:, c, :],
                        mybir.ActivationFunctionType.Square,
                        accum_out=d2c[:, c:c + 1],
                    )

                # ---- drift = (d2 > A) | (d2 < B) ---------------------------
                ga = small.tile([128, CH], mybir.dt.float32, tag="ga")
                gb = small.tile([128, CH], mybir.dt.float32, tag="gb")
                nc.vector.tensor_tensor(
                    out=ga[:], in0=d2c[:], in1=tabsel[:, :, 128],
                    op=mybir.AluOpType.is_gt,
                )
                nc.vector.tensor_tensor(
                    out=gb[:], in0=d2c[:], in1=tabsel[:, :, 129],
                    op=mybir.AluOpType.is_lt,
                )
                nc.vector.tensor_tensor(
                    out=driftacc[:, i * CH:(i + 1) * CH],
                    in0=ga[:], in1=gb[:], op=mybir.AluOpType.max,
                )
                if debug_taps:
                    nc.sync.dma_start(cls_dbg[i], cls4[:])
                    nc.sync.dma_start(d2_dbg[i], d2c[:])
                    nc.sync.dma_start(tab_dbg[i], tabsel[:])

            # ---- transpose [128, n_tiles*CH] -> token order and store ------
            ncols = n_tiles * CH
            tpsum = z2rp.tile([128, 128], mybir.dt.float32, tag="z2r")
            nc.tensor.transpose(tpsum[:ncols, :], driftacc[:, :ncols],
                                ident[:])
            drift_i = small.tile([128, 128], i32, tag="drifti")
            nc.vector.tensor_copy(out=drift_i[:ncols, :], in_=tpsum[:ncols, :])
            nc.sync.dma_start(
                drift_d.rearrange("(a b) -> a b", b=128),
                drift_i[:ncols, :],
            )

    nc.compile()
    return nc


def prep_inputs(x, noise, W1, b1, W2, b2, centroid, dis_median, mad,
                n_tiles=BS // TOK_TILE, n_cores=N_CORES):
    """Host-side preparation of per-core input maps."""
    bs = n_tiles * TOK_TILE
    x = np.asarray(x, dtype=np.float32)
    noise = np.asarray(noise, dtype=np.float32)
    W1 = np.asarray(W1, dtype=np.float32)
    b1 = np.asarray(b1, dtype=np.float32)
    W2 = np.asarray(W2, dtype=np.float32)
    b2 = np.asarray(b2, dtype=np.float32)
    centroid = np.asarray(centroid, dtype=np.float32)
    dis_median = np.asarray(dis_median, dtype=np.float32)
    mad = np.asarray(mad, dtype=np.float32)

    xn = x + noise

    W1s = np.ascontiguousarray(
        W1.reshape(KC1, 128, H).transpose(1, 0, 2)).astype(BF16)
    W2s = np.ascontiguousarray(
        W2.reshape(FC1, 128, D_LAT).transpose(1, 0, 2)).astype(BF16)
    b1s = np.ascontiguousarray(b1.reshape(FC1, 128).T)
    cTs = np.ascontiguousarray(centroid.T).astype(BF16)

    c2 = (centroid * centroid).sum(1)
    pre_f = PRE_SHIFT - 0.5 * c2 + centroid @ b2
    pre_hi = pre_f.astype(BF16)
    pre_lo = (pre_f - pre_hi.astype(np.float32)).astype(BF16)
    pre = np.ascontiguousarray(
        np.stack([pre_hi, pre_lo])[None, :, :])            # [1, 2, K]

    hi = dis_median + MAD_THRESHOLD * mad
    lo = dis_median - MAD_THRESHOLD * mad
    A = (hi * hi).astype(np.float32)
    Bv = np.where(lo > 0, lo * lo, -1.0).astype(np.float32)
    ctab = np.zeros((K, TAB_W), dtype=np.float32)
    ctab[:, :D_LAT] = centroid - b2[None, :]
    ctab[:, 128] = A
    ctab[:, 129] = Bv

    def shard_T(a, core):
        s = a[core * bs:(core + 1) * bs].astype(BF16)       # [bs, 512]
        sT = s.T                                            # [512, bs]
        blk = sT.reshape(KC1, 128, n_tiles, TOK_TILE).transpose(2, 0, 1, 3)
        return np.ascontiguousarray(blk)

    in_maps = []
    for core in range(n_cores):
        in_maps.append({
            "xT": shard_T(x, core),
            "xnT": shard_T(xn, core),
            "W1s": W1s,
            "W2s": W2s,
            "b1s": b1s,
            "cTs": cTs,
            "pre": pre,
            "ctab": ctab,
        })
    return in_maps


_BUILD_CACHE = {}


def kernel(x, noise, W1, b1, W2, b2, centroid, dis_median, mad):
    from concourse.bass_utils import run_bass_kernel_spmd

    nc = _BUILD_CACHE.get("nc")
    if nc is None:
        nc = _BUILD_CACHE["nc"] = build_program()
    in_maps = prep_inputs(x, noise, W1, b1, W2, b2, centroid,
                          dis_median, mad)
    res = run_bass_kernel_spmd(nc, in_maps, core_ids=list(range(N_CORES)))
    out = np.concatenate([r["drift"] for r in res.results])
    return out.astype(np.int32)
